# revision 25
# baseline (speedup 1.0000x reference)
"""DeepSeek-MoE SwiGLU expert layer on 8 TRN2 NeuronCores (expert parallelism).

Strategy (hardcoded for T=4096, D=1024, DFF=1408, E=8, K=2, 8 cores):
  - Expert parallelism: core e holds expert e's (Wg, Wu, Wd).
  - Dispatch happens at input-sharding time on the host: for each expert,
    gather the tokens routed to it (deduped via the combine matrix), pad to
    capacity C, and ship X^T to that core.  Shipping X transposed makes every
    matmul operand on-device natural-layout (contraction dim = partition
    dim), so the kernel needs zero transposes.
  - All matmul operands are bf16 (absmax rel err ~4e-3, well inside 2e-2);
    PSUM accumulation and the output stay fp32.
  - Weights are pre-tiled on the host so every DMA lands with >=512B
    contiguous per partition line (full DMA-engine rate, few descriptors):
      wgu[f, k, p, s, m] = (Wg if s==0 else Wu)[k*128+p, f*128+m]
      wd[do, p, k*128+m] = Wd[k*128+p, do*128+m]
      xt[k, p, c]        = x[token c, k*128+p]
  - Per core:  HT = silu(Wg^T @ XT) * (Wu^T @ XT)   [DFF, C]
               YT = Wd^T @ HT                        [D, C]
    Column blocks of 512 are processed outermost so stage 2 of block 0
    overlaps stage 1 of block 1; weights stay resident in SBUF.
  - DMA issue is spread across engine queues (weights on SP, x and output
    drains split between Activation and DVE) because each queue serializes
    its DMAs; outputs are DMA'd straight from PSUM.
  - Combine on host: out[idx_e] += (YT[:, :cnt]).T * combine_weight.
"""

import numpy as np
import ml_dtypes
from contextlib import ExitStack

import concourse.bass as bass
import concourse.tile as tile
from concourse import bacc, mybir
from concourse import bass_utils

T, D, DFF, E = 4096, 1024, 1408, 8
N_CORES = 8
P = 128
KD = D // P    # 8 k-tiles over D
KF = DFF // P  # 11 k-tiles over DFF
CT = 512       # matmul moving-operand width (one PSUM bank of fp32)

BF16 = ml_dtypes.bfloat16

_cache = {}


def _c_tiles(C):
    tiles = []
    off = 0
    while off < C:
        w = min(CT, C - off)
        tiles.append((off, w))
        off += w
    return tiles


def _emit_body(nc, pools, aps, C, warm=(0, 0, 0, 0)):
    bf = mybir.dt.bfloat16
    f32 = mybir.dt.float32
    ctiles = _c_tiles(C)
    NI = len(ctiles)
    xp, hp, wp, pp, sp, op = pools
    xt, wgu, wd, yt = aps
    Silu = mybir.ActivationFunctionType.Silu
    KH = KD // 2  # k-half for x loads

    # Warm-up filler matmuls on scratch data: executed in-order by the PE,
    # they absorb startup DMA waits and keep the clock ramp hot.  The counts
    # in `warm` are tuned against the timeline sim.
    w_sc = sp.tile([P, P], bf, tag="warml")
    r_sc = sp.tile([P, CT], bf, tag="warmr")
    if any(warm):
        nc.vector.memset(w_sc[:], 0)
        nc.vector.memset(r_sc[:], 0)

    def filler(n):
        for _ in range(n):
            ps_w = pp.tile([P, CT], f32, tag="warmp")
            nc.tensor.matmul(ps_w[:], lhsT=w_sc[:], rhs=r_sc[:],
                             start=True, stop=True)

    # --- f=0 weights: tiny k0 piece first so the first matmul starts as
    #     early as possible, then the k1-7 bulk.  Both on SP. ---
    wgu0_a = wp.tile([P, 1, 2, P], bf, tag="wgu0a", name="wgu0a")
    nc.sync.dma_start(out=wgu0_a[:],
                      in_=wgu[0, 0:1].rearrange("k p s m -> p k s m"))

    # --- x: ctile 0 ships k0 first (gates the first matmul), then k1-3 and
    #     k4-7; later ctiles are emitted mid-f-loop so they don't delay the
    #     first silus on ACT's serial queue. ---
    x_sb = {}
    x0_k0 = xp.tile([P, 1, ctiles[0][1]], bf, tag="x0k0", name="x0k0")
    nc.scalar.dma_start(out=x0_k0[:],
                        in_=xt[0:1, :, 0:ctiles[0][1]].rearrange(
                            "k p c -> p k c"))

    wgu0_b = wp.tile([P, KD - 1, 2, P], bf, tag="wgu0b", name="wgu0b")
    nc.sync.dma_start(out=wgu0_b[:],
                      in_=wgu[0, 1:KD].rearrange("k p s m -> p k s m"))
    wgu0 = (wgu0_a, wgu0_b)

    def emit_x(i, k_lo=0):
        c0, cw = ctiles[i]
        for h in range(2):
            klo, khi = (k_lo, KH) if h == 0 else (KH, KD)
            if klo >= khi:
                continue
            t = xp.tile([P, khi - klo, cw], bf, tag=f"x{i}h{h}",
                        name=f"x{i}h{h}")
            nc.scalar.dma_start(
                out=t[:],
                in_=xt[klo:khi, :, c0:c0 + cw].rearrange("k p c -> p k c"))
            x_sb[i, h] = t

    emit_x(0, k_lo=1)

    def x_view(i, k):
        if i == 0:
            if k == 0:
                return x0_k0[:, 0, :]
            if k < KH:
                return x_sb[0, 0][:, k - 1, :]
            return x_sb[0, 1][:, k - KH, :]
        return x_sb[i, k // KH][:, k % KH, :]

    # --- remaining weights, all resident, streamed on SP ---
    wgu_sl = {}
    for f in range(1, KF):
        t = wp.tile([P, KD, 2, P], bf, tag=f"wgu{f}", name=f"wgu{f}")
        nc.sync.dma_start(out=t[:], in_=wgu[f].rearrange("k p s m -> p k s m"))
        wgu_sl[f] = t
    wd_sl = {}
    for do in range(KD):
        t = wp.tile([P, KF * P], bf, tag=f"wd{do}", name=f"wd{do}")
        nc.sync.dma_start(out=t[:], in_=wd[do])
        wd_sl[do] = t

    def w1_view(f, k, s):
        if f == 0:
            if k == 0:
                return wgu0[0][:, 0, s, :]
            return wgu0[1][:, k - 1, s, :]
        return wgu_sl[f][:, k, s, :]

    h_sb = {}
    for i, (c0, cw) in enumerate(ctiles):
        # --- stage 1: HT[f, c] = silu(Wg^T XT) * (Wu^T XT) for this ctile ---
        h_sb[i] = hp.tile([P, KF, cw], bf, tag=f"h{i}", name=f"h_sb{i}")
        for f in range(KF):
            if i + 1 < NI and f == 3:
                emit_x(i + 1)
            ps_g = pp.tile([P, CT], f32, tag="psg")
            ps_u = pp.tile([P, CT], f32, tag="psu")
            ps = {0: ps_g, 1: ps_u}
            if i == 0 and f == 0:
                # consume k pieces in arrival order so the PE stays fed
                # while the later x/weight pieces are still landing, with
                # tuned filler bursts covering the remaining DMA waits
                filler(warm[0])
                order = [(0, range(1)), (1, range(1)), None,
                         (0, range(1, KH)), (1, range(1, KH)), None,
                         (0, range(KH, KD)), (1, range(KH, KD)), None]
                wi = 1
            else:
                order = [(0, range(KD)), (1, range(KD))]
                wi = None
            for item in order:
                if item is None:
                    filler(warm[wi])
                    wi += 1
                    continue
                s, ks = item
                for k in ks:
                    nc.tensor.matmul(ps[s][:, :cw], lhsT=w1_view(f, k, s),
                                     rhs=x_view(i, k),
                                     start=(k == 0), stop=(k == KD - 1))
            sg = sp.tile([P, CT], f32)
            nc.scalar.activation(sg[:, :cw], ps_g[:, :cw], Silu)
            nc.vector.tensor_mul(h_sb[i][:, f, :], sg[:, :cw], ps_u[:, :cw])

        # --- stage 2: YT[do, c] = Wd^T @ HT; DVE drains PSUM -> bf16 SBUF ---
        for do in range(KD):
            last = (i == NI - 1) and (do == KD - 1)
            row = yt[do * P:(do + 1) * P]
            if last and cw > P:
                # Split the last group into a wide + narrow pair sharing
                # each stationary (one Ldweights feeds both matmuls, so the
                # SEQ stays ahead).  The wide chunk's drain chain overlaps
                # the narrow chunk's matmuls; the final drain is small.
                wa = cw - P
                ps_a = pp.tile([P, CT], f32, tag="psy")
                ps_b = pp.tile([P, CT], f32, tag="psy")
                for k in range(KF):
                    lhs = wd_sl[do][:, k * P:(k + 1) * P]
                    nc.tensor.matmul(ps_a[:, :wa], lhsT=lhs,
                                     rhs=h_sb[i][:, k, :wa],
                                     start=(k == 0), stop=(k == KF - 1))
                    nc.tensor.matmul(ps_b[:, :P], lhsT=lhs,
                                     rhs=h_sb[i][:, k, wa:cw],
                                     start=(k == 0), stop=(k == KF - 1))
                y_a = op.tile([P, CT], bf)
                nc.vector.tensor_copy(y_a[:, :wa], ps_a[:, :wa])
                nc.sync.dma_start(out=row[:, c0:c0 + wa], in_=y_a[:, :wa])
                y_b = op.tile([P, P], bf, tag="yb")
                nc.vector.tensor_copy(y_b[:], ps_b[:, :P])
                nc.scalar.dma_start(out=row[:, c0 + wa:c0 + cw], in_=y_b[:])
                continue
            ps_y = pp.tile([P, CT], f32, tag="psy")
            for k in range(KF):
                nc.tensor.matmul(ps_y[:, :cw], lhsT=wd_sl[do][:, k * P:(k + 1) * P],
                                 rhs=h_sb[i][:, k, :],
                                 start=(k == 0), stop=(k == KF - 1))
            y_sb = op.tile([P, CT], bf)
            nc.vector.tensor_copy(y_sb[:, :cw], ps_y[:, :cw])
            # ctile 0 drains on ACT (SP still streaming weights);
            # later ctiles drain on SP (weights done by then).
            eng = nc.scalar if i == 0 else nc.sync
            eng.dma_start(out=row[:, c0:c0 + cw], in_=y_sb[:, :cw])


def _declare(nc, C):
    bf = mybir.dt.bfloat16
    xt = nc.dram_tensor("xt", [KD, P, C], bf, kind="ExternalInput").ap()
    wgu = nc.dram_tensor("wgu", [KF, KD, P, 2, P], bf,
                         kind="ExternalInput").ap()
    wd = nc.dram_tensor("wd", [KD, P, KF * P], bf, kind="ExternalInput").ap()
    yt = nc.dram_tensor("yt", [D, C], bf, kind="ExternalOutput").ap()
    return (xt, wgu, wd, yt)


def _pools(tc, ctx):
    xp = ctx.enter_context(tc.tile_pool(name="xt_p", bufs=1))
    hp = ctx.enter_context(tc.tile_pool(name="ht_p", bufs=1))
    wp = ctx.enter_context(tc.tile_pool(name="w_p", bufs=1))
    pp = ctx.enter_context(tc.tile_pool(name="ps_p", bufs=2, space="PSUM"))
    sp = ctx.enter_context(tc.tile_pool(name="sg_p", bufs=2))
    op = ctx.enter_context(tc.tile_pool(name="y_p", bufs=3))
    return (xp, hp, wp, pp, sp, op)


WARM = (0, 0, 0, 0)


def _build(C, warm=None):
    warm = WARM if warm is None else warm
    key = ("plain", C, warm)
    if key in _cache:
        return _cache[key]
    nc = bacc.Bacc("TRN2", target_bir_lowering=False, debug=False,
                   num_devices=N_CORES)
    aps = _declare(nc, C)
    with tile.TileContext(nc) as tc, ExitStack() as ctx:
        pools = _pools(tc, ctx)
        _emit_body(nc, pools, aps, C, warm=warm)
    nc.compile()
    _cache[key] = nc
    return nc


def _build_loop(C):
    """Benchmark variant: repeat the body niter times (runtime input)."""
    key = ("loop", C)
    if key in _cache:
        return _cache[key]
    nc = bacc.Bacc("TRN2", target_bir_lowering=False, debug=False,
                   num_devices=N_CORES)
    aps = _declare(nc, C)
    n_ap = nc.dram_tensor("niter", [1, 1], mybir.dt.uint32,
                          kind="ExternalInput").ap()
    with tile.TileContext(nc) as tc, ExitStack() as ctx:
        cpool = ctx.enter_context(tc.tile_pool(name="c_p", bufs=1))
        pools = _pools(tc, ctx)
        n_sb = cpool.tile([1, 1], mybir.dt.uint32)
        nc.sync.dma_start(out=n_sb[:], in_=n_ap[:])
        with tc.tile_critical():
            tmp = nc.alloc_registers("niter_regs")
            nc.regs_load(tmp, n_sb[0:1, 0:1])
            n_val = nc.snap(tmp, donate=True, min_val=0, max_val=1 << 20)
        with tc.For_i(0, n_val, 1, hint_engines=(mybir.EngineType.PE,)):
            _emit_body(nc, pools, aps, C, warm=WARM)
    nc.compile()
    _cache[key] = nc
    return nc


def _dispatch(x, topk_weights, topk_indices, num_experts):
    """Host-side routing: combine matrix + per-expert token index lists."""
    T_, _ = x.shape
    E_ = int(num_experts)
    ti = np.asarray(topk_indices).astype(np.int64)
    tw = np.asarray(topk_weights).astype(np.float32)
    combine = np.zeros((T_, E_), np.float32)
    np.add.at(combine, (np.arange(T_)[:, None], ti), tw)
    idxs = [np.nonzero(combine[:, e])[0] for e in range(E_)]
    return combine, idxs


def _capacity(idxs):
    maxc = max((len(i) for i in idxs), default=0)
    return max(64, ((maxc + 31) // 32) * 32)


def _in_maps(x, Wg, Wu, Wd, idxs, C):
    maps = []
    D_ = x.shape[1]
    for e in range(len(idxs)):
        xt_e = np.zeros((D_, C), BF16)
        n = len(idxs[e])
        if n:
            xt_e[:, :n] = x[idxs[e]].astype(BF16).T
        wg4 = np.asarray(Wg[e], BF16).reshape(KD, P, KF, P)
        wu4 = np.asarray(Wu[e], BF16).reshape(KD, P, KF, P)
        wgu = np.ascontiguousarray(
            np.stack([wg4, wu4], axis=3).transpose(2, 0, 1, 3, 4))
        wd4 = np.asarray(Wd[e], BF16).reshape(KF, P, KD, P)
        wdt = np.ascontiguousarray(
            wd4.transpose(2, 1, 0, 3).reshape(KD, P, KF * P))
        maps.append({
            "xt": xt_e.reshape(KD, P, C),
            "wgu": wgu,
            "wd": wdt,
        })
    return maps


def kernel(x, Wg, Wu, Wd, topk_weights, topk_indices, num_experts):
    x = np.asarray(x, np.float32)
    Wg = np.asarray(Wg, np.float32)
    Wu = np.asarray(Wu, np.float32)
    Wd = np.asarray(Wd, np.float32)
    T_, D_ = x.shape

    combine, idxs = _dispatch(x, topk_weights, topk_indices, num_experts)
    C = _capacity(idxs)

    nc = _build(C)
    res = bass_utils.run_bass_kernel_spmd(nc, _in_maps(x, Wg, Wu, Wd, idxs, C),
                                          list(range(N_CORES)))

    out = np.zeros((T_, D_), np.float32)
    for e in range(len(idxs)):
        n = len(idxs[e])
        if n:
            ye = res.results[e]["yt"][:, :n].T.astype(np.float32)
            out[idxs[e]] += ye * combine[idxs[e], e][:, None]
    return out


# revision 26
# speedup vs baseline: 1.4284x; 1.4284x over previous
"""DeepSeek-MoE SwiGLU expert layer on 8 TRN2 NeuronCores (expert parallelism).

Strategy (hardcoded for T=4096, D=1024, DFF=1408, E=8, K=2, 8 cores):
  - Expert parallelism: core e holds expert e's (Wg, Wu, Wd).
  - Dispatch happens at input-sharding time on the host: for each expert,
    gather the tokens routed to it (deduped via the combine matrix), pad to
    capacity C, and ship X^T to that core.  Shipping X transposed makes every
    matmul operand on-device natural-layout (contraction dim = partition
    dim), so the kernel needs zero transposes.
  - All matmul operands are bf16 (absmax rel err ~4e-3, well inside 2e-2);
    PSUM accumulation and the output stay fp32.
  - Weights are pre-tiled on the host so every DMA lands with >=512B
    contiguous per partition line (full DMA-engine rate, few descriptors):
      wgu[f, k, p, s, m] = (Wg if s==0 else Wu)[k*128+p, f*128+m]
      wd[do, p, k*128+m] = Wd[k*128+p, do*128+m]
      xt[k, p, c]        = x[token c, k*128+p]
  - Per core:  HT = silu(Wg^T @ XT) * (Wu^T @ XT)   [DFF, C]
               YT = Wd^T @ HT                        [D, C]
    Column blocks of 512 are processed outermost so stage 2 of block 0
    overlaps stage 1 of block 1; weights stay resident in SBUF.
  - DMA issue is spread across engine queues (weights on SP, x and output
    drains split between Activation and DVE) because each queue serializes
    its DMAs; outputs are DMA'd straight from PSUM.
  - Combine on host: out[idx_e] += (YT[:, :cnt]).T * combine_weight.
"""

import numpy as np
import ml_dtypes
from contextlib import ExitStack

import concourse.bass as bass
import concourse.tile as tile
from concourse import bacc, mybir
from concourse import bass_utils

T, D, DFF, E = 4096, 1024, 1408, 8
N_CORES = 8
P = 128
KD = D // P    # 8 k-tiles over D
KF = DFF // P  # 11 k-tiles over DFF
CT = 512       # matmul moving-operand width (one PSUM bank of fp32)

BF16 = ml_dtypes.bfloat16

_cache = {}


def _c_tiles(C):
    tiles = []
    off = 0
    while off < C:
        w = min(CT, C - off)
        tiles.append((off, w))
        off += w
    return tiles


def _emit_body(nc, pools, aps, C, warm=(0, 0, 0, 0)):
    bf = mybir.dt.bfloat16
    f32 = mybir.dt.float32
    ctiles = _c_tiles(C)
    NI = len(ctiles)
    xp, hp, wp, pp, sp, op = pools
    xt, wgu, wd, yt = aps
    Silu = mybir.ActivationFunctionType.Silu
    KH = KD // 2  # k-half for x loads

    # Warm-up filler matmuls on scratch data: executed in-order by the PE,
    # they absorb startup DMA waits and keep the clock ramp hot.  The counts
    # in `warm` are tuned against the timeline sim.
    w_sc = sp.tile([P, P], bf, tag="warml")
    r_sc = sp.tile([P, CT], bf, tag="warmr")
    if any(warm):
        nc.vector.memset(w_sc[:], 0)
        nc.vector.memset(r_sc[:], 0)

    def filler(n):
        for _ in range(n):
            ps_w = pp.tile([P, CT], f32, tag="warmp")
            nc.tensor.matmul(ps_w[:], lhsT=w_sc[:], rhs=r_sc[:],
                             start=True, stop=True)

    # --- f=0 weights: tiny k0 piece first so the first matmul starts as
    #     early as possible, then the k1-7 bulk.  Both on SP. ---
    wgu0_a = wp.tile([P, 1, 2, P], bf, tag="wgu0a", name="wgu0a")
    nc.sync.dma_start(out=wgu0_a[:],
                      in_=wgu[0, 0:1].rearrange("k p s m -> p k s m"))

    # --- x: ctile 0 ships k0 first (gates the first matmul), then k1-3 and
    #     k4-7; later ctiles are emitted mid-f-loop so they don't delay the
    #     first silus on ACT's serial queue. ---
    x_sb = {}
    x0_k0 = xp.tile([P, 1, ctiles[0][1]], bf, tag="x0k0", name="x0k0")
    nc.scalar.dma_start(out=x0_k0[:],
                        in_=xt[0:1, :, 0:ctiles[0][1]].rearrange(
                            "k p c -> p k c"))

    wgu0_b = wp.tile([P, KD - 1, 2, P], bf, tag="wgu0b", name="wgu0b")
    nc.sync.dma_start(out=wgu0_b[:],
                      in_=wgu[0, 1:KD].rearrange("k p s m -> p k s m"))
    wgu0 = (wgu0_a, wgu0_b)

    def emit_x(i, k_lo=0):
        c0, cw = ctiles[i]
        for h in range(2):
            klo, khi = (k_lo, KH) if h == 0 else (KH, KD)
            if klo >= khi:
                continue
            t = xp.tile([P, khi - klo, cw], bf, tag=f"x{i}h{h}",
                        name=f"x{i}h{h}")
            nc.scalar.dma_start(
                out=t[:],
                in_=xt[klo:khi, :, c0:c0 + cw].rearrange("k p c -> p k c"))
            x_sb[i, h] = t

    emit_x(0, k_lo=1)

    def x_view(i, k):
        if i == 0:
            if k == 0:
                return x0_k0[:, 0, :]
            if k < KH:
                return x_sb[0, 0][:, k - 1, :]
            return x_sb[0, 1][:, k - KH, :]
        return x_sb[i, k // KH][:, k % KH, :]

    # --- remaining weights, all resident, streamed on SP ---
    wgu_sl = {}
    for f in range(1, KF):
        t = wp.tile([P, KD, 2, P], bf, tag=f"wgu{f}", name=f"wgu{f}")
        nc.sync.dma_start(out=t[:], in_=wgu[f].rearrange("k p s m -> p k s m"))
        wgu_sl[f] = t
    wd_sl = {}
    for do in range(KD):
        t = wp.tile([P, KF * P], bf, tag=f"wd{do}", name=f"wd{do}")
        nc.sync.dma_start(out=t[:], in_=wd[do])
        wd_sl[do] = t

    def w1_view(f, k, s):
        if f == 0:
            if k == 0:
                return wgu0[0][:, 0, s, :]
            return wgu0[1][:, k - 1, s, :]
        return wgu_sl[f][:, k, s, :]

    h_sb = {}
    for i, (c0, cw) in enumerate(ctiles):
        # --- stage 1: HT[f, c] = silu(Wg^T XT) * (Wu^T XT) for this ctile ---
        h_sb[i] = hp.tile([P, KF, cw], bf, tag=f"h{i}", name=f"h_sb{i}")
        for f in range(KF):
            if i + 1 < NI and f == 3:
                emit_x(i + 1)
            ps_g = pp.tile([P, CT], f32, tag="psg")
            ps_u = pp.tile([P, CT], f32, tag="psu")
            ps = {0: ps_g, 1: ps_u}
            if i == 0 and f == 0:
                # consume k pieces in arrival order so the PE stays fed
                # while the later x/weight pieces are still landing, with
                # tuned filler bursts covering the remaining DMA waits
                filler(warm[0])
                order = [(0, range(1)), (1, range(1)), None,
                         (0, range(1, KH)), (1, range(1, KH)), None,
                         (0, range(KH, KD)), (1, range(KH, KD)), None]
                wi = 1
            else:
                order = [(0, range(KD)), (1, range(KD))]
                wi = None
            for item in order:
                if item is None:
                    filler(warm[wi])
                    wi += 1
                    continue
                s, ks = item
                for k in ks:
                    nc.tensor.matmul(ps[s][:, :cw], lhsT=w1_view(f, k, s),
                                     rhs=x_view(i, k),
                                     start=(k == 0), stop=(k == KD - 1))
            sg = sp.tile([P, CT], f32)
            nc.scalar.activation(sg[:, :cw], ps_g[:, :cw], Silu)
            nc.vector.tensor_mul(h_sb[i][:, f, :], sg[:, :cw], ps_u[:, :cw])

        # --- stage 2: YT[do, c] = Wd^T @ HT; DVE drains PSUM -> bf16 SBUF ---
        for do in range(KD):
            last = (i == NI - 1) and (do == KD - 1)
            row = yt[do * P:(do + 1) * P]
            if last and cw > P:
                # Split the last group into a wide + narrow pair sharing
                # each stationary (one Ldweights feeds both matmuls, so the
                # SEQ stays ahead).  The wide chunk's drain chain overlaps
                # the narrow chunk's matmuls; the final drain is small.
                wa = cw - P
                ps_a = pp.tile([P, CT], f32, tag="psy")
                ps_b = pp.tile([P, CT], f32, tag="psy")
                for k in range(KF):
                    lhs = wd_sl[do][:, k * P:(k + 1) * P]
                    nc.tensor.matmul(ps_a[:, :wa], lhsT=lhs,
                                     rhs=h_sb[i][:, k, :wa],
                                     start=(k == 0), stop=(k == KF - 1))
                    nc.tensor.matmul(ps_b[:, :P], lhsT=lhs,
                                     rhs=h_sb[i][:, k, wa:cw],
                                     start=(k == 0), stop=(k == KF - 1))
                y_b = op.tile([P, P], bf, tag="yb")
                nc.vector.tensor_copy(y_b[:], ps_b[:, :P])
                nc.scalar.dma_start(out=row[:, c0 + wa:c0 + cw], in_=y_b[:])
                y_a = op.tile([P, CT], bf)
                nc.vector.tensor_copy(y_a[:, :wa], ps_a[:, :wa])
                nc.sync.dma_start(out=row[:, c0:c0 + wa], in_=y_a[:, :wa])
                continue
            ps_y = pp.tile([P, CT], f32, tag="psy")
            for k in range(KF):
                nc.tensor.matmul(ps_y[:, :cw], lhsT=wd_sl[do][:, k * P:(k + 1) * P],
                                 rhs=h_sb[i][:, k, :],
                                 start=(k == 0), stop=(k == KF - 1))
            y_sb = op.tile([P, CT], bf)
            nc.vector.tensor_copy(y_sb[:, :cw], ps_y[:, :cw])
            # ctile 0 drains on ACT (SP still streaming weights);
            # later ctiles drain on SP (weights done by then).
            eng = nc.scalar if i == 0 else nc.sync
            eng.dma_start(out=row[:, c0:c0 + cw], in_=y_sb[:, :cw])


def _declare(nc, C):
    bf = mybir.dt.bfloat16
    xt = nc.dram_tensor("xt", [KD, P, C], bf, kind="ExternalInput").ap()
    wgu = nc.dram_tensor("wgu", [KF, KD, P, 2, P], bf,
                         kind="ExternalInput").ap()
    wd = nc.dram_tensor("wd", [KD, P, KF * P], bf, kind="ExternalInput").ap()
    yt = nc.dram_tensor("yt", [D, C], bf, kind="ExternalOutput").ap()
    return (xt, wgu, wd, yt)


def _pools(tc, ctx):
    xp = ctx.enter_context(tc.tile_pool(name="xt_p", bufs=1))
    hp = ctx.enter_context(tc.tile_pool(name="ht_p", bufs=1))
    wp = ctx.enter_context(tc.tile_pool(name="w_p", bufs=1))
    pp = ctx.enter_context(tc.tile_pool(name="ps_p", bufs=2, space="PSUM"))
    sp = ctx.enter_context(tc.tile_pool(name="sg_p", bufs=2))
    op = ctx.enter_context(tc.tile_pool(name="y_p", bufs=3))
    return (xp, hp, wp, pp, sp, op)


WARM = (0, 0, 0, 0)


def _build(C, warm=None):
    warm = WARM if warm is None else warm
    key = ("plain", C, warm)
    if key in _cache:
        return _cache[key]
    nc = bacc.Bacc("TRN2", target_bir_lowering=False, debug=False,
                   num_devices=N_CORES)
    aps = _declare(nc, C)
    with tile.TileContext(nc) as tc, ExitStack() as ctx:
        pools = _pools(tc, ctx)
        _emit_body(nc, pools, aps, C, warm=warm)
    nc.compile()
    _cache[key] = nc
    return nc


def _build_loop(C):
    """Benchmark variant: repeat the body niter times (runtime input)."""
    key = ("loop", C)
    if key in _cache:
        return _cache[key]
    nc = bacc.Bacc("TRN2", target_bir_lowering=False, debug=False,
                   num_devices=N_CORES)
    aps = _declare(nc, C)
    n_ap = nc.dram_tensor("niter", [1, 1], mybir.dt.uint32,
                          kind="ExternalInput").ap()
    with tile.TileContext(nc) as tc, ExitStack() as ctx:
        cpool = ctx.enter_context(tc.tile_pool(name="c_p", bufs=1))
        pools = _pools(tc, ctx)
        n_sb = cpool.tile([1, 1], mybir.dt.uint32)
        nc.sync.dma_start(out=n_sb[:], in_=n_ap[:])
        with tc.tile_critical():
            tmp = nc.alloc_registers("niter_regs")
            nc.regs_load(tmp, n_sb[0:1, 0:1])
            n_val = nc.snap(tmp, donate=True, min_val=0, max_val=1 << 20)
        with tc.For_i(0, n_val, 1, hint_engines=(mybir.EngineType.PE,)):
            _emit_body(nc, pools, aps, C, warm=WARM)
    nc.compile()
    _cache[key] = nc
    return nc


def _dispatch(x, topk_weights, topk_indices, num_experts):
    """Host-side routing: combine matrix + per-expert token index lists."""
    T_, _ = x.shape
    E_ = int(num_experts)
    ti = np.asarray(topk_indices).astype(np.int64)
    tw = np.asarray(topk_weights).astype(np.float32)
    combine = np.zeros((T_, E_), np.float32)
    np.add.at(combine, (np.arange(T_)[:, None], ti), tw)
    idxs = [np.nonzero(combine[:, e])[0] for e in range(E_)]
    return combine, idxs


def _capacity(idxs):
    maxc = max((len(i) for i in idxs), default=0)
    return max(64, ((maxc + 31) // 32) * 32)


def _in_maps(x, Wg, Wu, Wd, idxs, C):
    maps = []
    D_ = x.shape[1]
    for e in range(len(idxs)):
        xt_e = np.zeros((D_, C), BF16)
        n = len(idxs[e])
        if n:
            xt_e[:, :n] = x[idxs[e]].astype(BF16).T
        wg4 = np.asarray(Wg[e], BF16).reshape(KD, P, KF, P)
        wu4 = np.asarray(Wu[e], BF16).reshape(KD, P, KF, P)
        wgu = np.ascontiguousarray(
            np.stack([wg4, wu4], axis=3).transpose(2, 0, 1, 3, 4))
        wd4 = np.asarray(Wd[e], BF16).reshape(KF, P, KD, P)
        wdt = np.ascontiguousarray(
            wd4.transpose(2, 1, 0, 3).reshape(KD, P, KF * P))
        maps.append({
            "xt": xt_e.reshape(KD, P, C),
            "wgu": wgu,
            "wd": wdt,
        })
    return maps


def kernel(x, Wg, Wu, Wd, topk_weights, topk_indices, num_experts):
    x = np.asarray(x, np.float32)
    Wg = np.asarray(Wg, np.float32)
    Wu = np.asarray(Wu, np.float32)
    Wd = np.asarray(Wd, np.float32)
    T_, D_ = x.shape

    combine, idxs = _dispatch(x, topk_weights, topk_indices, num_experts)
    C = _capacity(idxs)

    nc = _build(C)
    res = bass_utils.run_bass_kernel_spmd(nc, _in_maps(x, Wg, Wu, Wd, idxs, C),
                                          list(range(N_CORES)))

    out = np.zeros((T_, D_), np.float32)
    for e in range(len(idxs)):
        n = len(idxs[e])
        if n:
            ye = res.results[e]["yt"][:, :n].T.astype(np.float32)
            out[idxs[e]] += ye * combine[idxs[e], e][:, None]
    return out
